# revision 1
# baseline (speedup 1.0000x reference)
"""Trainium2 Bass kernel for temporal attention (nn_Attention_4423816315129).

Per 128-token tile (= 4 temporal groups x 32 positions, one batch):
  xT    <- one DMA load of host pre-transposed x (bf16, dim on partitions)
  q,k,v <- PE matmuls accumulating 4 dim-blocks (PSUM fp32)
  rope  <- DVE: t*cos + swap(t)*sin_signed (swap = reversed-pair AP view)
  qT,kT <- PE transposes (128x128 blocks), ACT copy to SBUF bf16
  sim   <- per head ONE full matmul (K=64, M=128, N=128): all (group,group')
           pairs; cross-group entries killed by a -1e30 bias added on PE via
           identity @ pb_full (also encodes pos_bias and the focus-present
           diagonal mask), so exp() zeroes them exactly
  softmax (no max-subtract; logits are small): ACT exp, DVE segmented row
           sums + reciprocal; division folded into attention weights
  attnT <- per head PE transpose of the (128x128) attention block
  y     <- per head matmul lhsT=attnT rhs=v slice -> y natural (128 x 512)
  yT    <- PE transposes, then out = yT @ w_out blocks, fp32 copy, DMA store

All matmuls write full-partition contiguous PSUM regions (partition-sliced
multi-matmul PSUM writes are fatal on this toolchain/HW).
Sharding: hw axis split across 8 cores, pure data parallel.
"""

import numpy as np
import ml_dtypes

import concourse.bass as bass
from concourse import bacc
import concourse.mybir as mybir
import concourse.tile as tile
from concourse.bass import ts
from concourse.masks import make_identity
from concourse.bass_utils import run_bass_kernel_spmd

HEADS = 8
DIM_HEAD = 64
B = 2
HW = 1024
N = 32
DIM = 512
N_CORES = 8
HW_SHARD = HW // N_CORES            # 128
TOK = B * HW_SHARD * N              # 8192 tokens per core
TILE_T = 128
N_TILES = TOK // TILE_T             # 64
BF16 = mybir.dt.bfloat16
F32 = mybir.dt.float32


def build_nc(mask_flags, n_tiles=N_TILES, repeat=1, stage='full'):
    nc = bacc.Bacc("TRN2", target_bir_lowering=False)

    x_d = nc.dram_tensor("x", [TOK, DIM], BF16, kind="ExternalInput")
    wq_d = nc.dram_tensor("wq", [DIM, DIM], BF16, kind="ExternalInput")
    wkv_d = nc.dram_tensor("wkv", [DIM, 2 * DIM], BF16, kind="ExternalInput")
    wo_d = nc.dram_tensor("wo", [DIM, DIM], BF16, kind="ExternalInput")
    cos_d = nc.dram_tensor("cosb", [TILE_T, DIM], BF16, kind="ExternalInput")
    sin_d = nc.dram_tensor("sinb", [TILE_T, DIM], BF16, kind="ExternalInput")
    pb_d = nc.dram_tensor("pbias", [TILE_T, B * 2048], BF16, kind="ExternalInput")
    out_d = nc.dram_tensor("out", [TOK, DIM], F32, kind="ExternalOutput")

    Exp = mybir.ActivationFunctionType.Exp
    Add = mybir.AluOpType.add

    with tile.TileContext(nc) as tc:
        with (
            tc.tile_pool(name="const", bufs=1) as cpool,
            tc.tile_pool(name="xt", bufs=N_TILES) as xtpool,
            tc.tile_pool(name="work", bufs=4) as wpool,
            tc.tile_pool(name="small", bufs=3) as spool,
            tc.tile_pool(name="ps", bufs=8, space="PSUM") as ps,
        ):
            wq_sb = cpool.tile([128, 4, DIM], BF16)
            nc.gpsimd.dma_start(wq_sb[:], wq_d.ap().rearrange("(kb p) c -> p kb c", p=128))
            wkv_sb = cpool.tile([128, 4, 2 * DIM], BF16)
            nc.gpsimd.dma_start(wkv_sb[:], wkv_d.ap().rearrange("(kb p) c -> p kb c", p=128))
            wo_sb = cpool.tile([128, 4, DIM], BF16)
            nc.gpsimd.dma_start(wo_sb[:], wo_d.ap().rearrange("(kb p) c -> p kb c", p=128))
            cos_sb = cpool.tile([TILE_T, DIM], BF16)
            nc.gpsimd.dma_start(cos_sb[:], cos_d.ap())
            sin_sb = cpool.tile([TILE_T, DIM], BF16)
            nc.gpsimd.dma_start(sin_sb[:], sin_d.ap())
            pb_sb = cpool.tile([TILE_T, B * 2048], BF16)
            nc.gpsimd.dma_start(pb_sb[:], pb_d.ap())
            ident = cpool.tile([128, 128], BF16)
            make_identity(nc, ident[:])

            def rope(dst, src_bf):
                t1 = wpool.tile([TILE_T, DIM], BF16, tag="rope_t1")
                nc.vector.tensor_mul(t1[:], src_bf[:], cos_sb[:])
                t2 = wpool.tile([TILE_T, DIM], BF16, tag="rope_t2")
                sw = src_bf[:].rearrange("p (c two) -> p c two", two=2)[:, :, ::-1]
                nc.vector.tensor_mul(
                    t2[:].rearrange("p (c two) -> p c two", two=2),
                    sw,
                    sin_sb[:].rearrange("p (c two) -> p c two", two=2),
                )
                nc.vector.tensor_add(dst[:], t1[:], t2[:])

            for rep in range(repeat):
              for tt in range(n_tiles):
                b = tt // max(1, n_tiles // B)

                xT = xtpool.tile([128, DIM], BF16)
                nc.sync.dma_start(xT[:], x_d.ap()[ts(tt, TILE_T), :])

                # ---- projections ----
                v_ps = ps.tile([TILE_T, DIM], F32, tag="ps")
                for kb in range(4):
                    nc.tensor.matmul(v_ps[:], xT[:, ts(kb, 128)], wkv_sb[:, kb, DIM:],
                                     start=(kb == 0), stop=(kb == 3))
                v_sb = wpool.tile([TILE_T, DIM], BF16, tag="v_sb")
                nc.scalar.copy(v_sb[:], v_ps[:])

                q_ps = ps.tile([TILE_T, DIM], F32, tag="ps")
                k_ps = ps.tile([TILE_T, DIM], F32, tag="ps")
                for kb in range(4):
                    nc.tensor.matmul(q_ps[:], xT[:, ts(kb, 128)], wq_sb[:, kb, :],
                                     start=(kb == 0), stop=(kb == 3))
                for kb in range(4):
                    nc.tensor.matmul(k_ps[:], xT[:, ts(kb, 128)], wkv_sb[:, kb, :DIM],
                                     start=(kb == 0), stop=(kb == 3))

                q_bf = wpool.tile([TILE_T, DIM], BF16, tag="q_bf")
                nc.scalar.copy(q_bf[:], q_ps[:])
                k_bf = wpool.tile([TILE_T, DIM], BF16, tag="k_bf")
                nc.scalar.copy(k_bf[:], k_ps[:])

                q_r = wpool.tile([TILE_T, DIM], BF16, tag="q_r")
                rope(q_r, q_bf)
                k_r = wpool.tile([TILE_T, DIM], BF16, tag="k_r")
                rope(k_r, k_bf)

                # ---- qT, kT per head: (64 d x 128 tok), base partition 0
                # (mixing matmul operand base partitions is fatal on this HW) ----
                qT_ps = ps.tile([64, 2 * DIM], BF16, tag="ps")
                kT_ps = ps.tile([64, 2 * DIM], BF16, tag="ps")
                for h in range(HEADS):
                    nc.tensor.transpose(qT_ps[:, ts(h, 128)], q_r[:, ts(h, 64)], ident[:])
                for h in range(HEADS):
                    nc.tensor.transpose(kT_ps[:, ts(h, 128)], k_r[:, ts(h, 64)], ident[:])
                qT_sb = wpool.tile([64, 2 * DIM], BF16, tag="qT_sb")
                nc.scalar.copy(qT_sb[:], qT_ps[:])
                kT_sb = wpool.tile([64, 2 * DIM], BF16, tag="kT_sb")
                nc.scalar.copy(kT_sb[:], kT_ps[:])

                # ---- sim: per head, all group pairs (128x128) ----
                sim_ps = [ps.tile([TILE_T, DIM], F32, tag="ps", name=f"sim_ps{i}") for i in range(2)]
                for h in range(HEADS):
                    nc.tensor.matmul(
                        sim_ps[h // 4][:, ts(h % 4, 128)],
                        qT_sb[:, ts(h, 128)],
                        kT_sb[:, ts(h, 128)],
                        start=(h % 4 == 0), stop=False, skip_group_check=True,
                    )
                # + pos_bias/mask via identity @ pb_full (kills cross-group)
                for h in range(HEADS):
                    cb = b * 2048 + (h // 4) * 1024 + (h % 4) * 128
                    nc.tensor.matmul(
                        sim_ps[h // 4][:, ts(h % 4, 128)],
                        ident[:],
                        pb_sb[:, cb:cb + 128],
                        start=False, stop=(h % 4 == 3), skip_group_check=True,
                    )

                if stage == 'sim':
                    o_sb = wpool.tile([TILE_T, DIM], F32, tag="o_sb")
                    nc.scalar.copy(o_sb[:], sim_ps[0][:])
                    nc.scalar.dma_start(out_d.ap()[ts(tt, TILE_T), :], o_sb[:])
                    continue
                # ---- softmax ----
                e_sb = wpool.tile([TILE_T, 2 * DIM], BF16, tag="e_sb")
                nc.scalar.activation(e_sb[:, :DIM], sim_ps[0][:], Exp)
                nc.scalar.activation(e_sb[:, DIM:], sim_ps[1][:], Exp)
                sums = spool.tile([TILE_T, HEADS], F32, tag="sums")
                nc.vector.tensor_reduce(
                    sums[:], e_sb[:].rearrange("p (h j) -> p h j", j=128),
                    axis=mybir.AxisListType.X, op=Add,
                )
                rcp = spool.tile([TILE_T, HEADS], F32, tag="rcp")
                nc.vector.reciprocal(rcp[:], sums[:])
                attn = wpool.tile([TILE_T, 2 * DIM], BF16, tag="attn")
                nc.vector.tensor_mul(
                    attn[:].rearrange("p (h j) -> p h j", j=128),
                    e_sb[:].rearrange("p (h j) -> p h j", j=128),
                    rcp[:].rearrange("p h -> p h ()").broadcast_to((TILE_T, HEADS, 128)),
                )

                if stage == 'attn':
                    o_sb = wpool.tile([TILE_T, DIM], F32, tag="o_sb")
                    nc.scalar.copy(o_sb[:], attn[:, :DIM])
                    nc.scalar.dma_start(out_d.ap()[ts(tt, TILE_T), :], o_sb[:])
                    continue
                # ---- attnT per head (PE transpose), then y = attnT.T @ v ----
                aT_ps = [ps.tile([TILE_T, DIM], BF16, tag="ps", name=f"aT_ps{i}") for i in range(2)]
                for h in range(HEADS):
                    nc.tensor.transpose(aT_ps[h // 4][:, ts(h % 4, 128)],
                                        attn[:, ts(h, 128)], ident[:])
                aT_sb = wpool.tile([TILE_T, 2 * DIM], BF16, tag="aT_sb")
                nc.scalar.copy(aT_sb[:, :DIM], aT_ps[0][:])
                nc.scalar.copy(aT_sb[:, DIM:], aT_ps[1][:])

                y_ps = ps.tile([TILE_T, DIM], F32, tag="ps")
                for h in range(HEADS):
                    nc.tensor.matmul(y_ps[:, ts(h, 64)],
                                     aT_sb[:, ts(h, 128)], v_sb[:, ts(h, 64)],
                                     start=(h == 0), stop=(h == 7), skip_group_check=True)
                if stage == 'y':
                    o_sb = wpool.tile([TILE_T, DIM], F32, tag="o_sb")
                    nc.scalar.copy(o_sb[:], y_ps[:])
                    nc.scalar.dma_start(out_d.ap()[ts(tt, TILE_T), :], o_sb[:])
                    continue
                y_sb = wpool.tile([TILE_T, DIM], BF16, tag="y_sb")
                nc.scalar.copy(y_sb[:], y_ps[:])

                yT_ps = ps.tile([TILE_T, DIM], BF16, tag="ps")
                for hb in range(4):
                    nc.tensor.transpose(yT_ps[:, ts(hb, 128)], y_sb[:, ts(hb, 128)], ident[:])
                yT_sb = wpool.tile([TILE_T, DIM], BF16, tag="yT_sb")
                nc.scalar.copy(yT_sb[:], yT_ps[:])

                # ---- out projection ----
                o_ps = ps.tile([TILE_T, DIM], F32, tag="ps")
                for hb in range(4):
                    nc.tensor.matmul(o_ps[:], yT_sb[:, ts(hb, 128)], wo_sb[:, hb, :],
                                     start=(hb == 0), stop=(hb == 3))
                o_sb = wpool.tile([TILE_T, DIM], F32, tag="o_sb")
                nc.scalar.copy(o_sb[:], o_ps[:])
                nc.scalar.dma_start(out_d.ap()[ts(tt, TILE_T), :], o_sb[:])

    nc.compile()
    return nc


def _host_tables(pos_bias, focus_present_mask, inv_freq):
    pos = np.arange(N, dtype=np.float32)
    freqs = pos[:, None] * np.asarray(inv_freq, np.float32)[None, :]
    fr2 = np.repeat(freqs, 2, axis=-1)
    cos = np.cos(fr2)
    sin = np.sin(fr2)
    sign = np.tile(np.array([-1.0, 1.0], np.float32), DIM_HEAD // 2)
    sin_signed = sin * sign[None, :]
    cos_t = np.tile(cos, (4, HEADS)).astype(ml_dtypes.bfloat16)
    sin_t = np.tile(sin_signed, (4, HEADS)).astype(ml_dtypes.bfloat16)

    pb = np.asarray(pos_bias, np.float32)
    mask = np.asarray(focus_present_mask)
    eye = np.eye(N, dtype=bool)
    # pb_full[(g,i), b*2048 + (h//4)*1024 + (h%4)*128 + g'*32 + j]
    pb_t = np.full((TILE_T, B * 2048), -1e30, np.float32)
    for b in range(B):
        base = pb.copy()                       # (H, 32, 32)
        if mask[b]:
            m = np.full_like(base, -1e30)
            m[:, eye] = base[:, eye]
            base = m
        for h in range(HEADS):
            for g in range(4):
                col = b * 2048 + (h // 4) * 1024 + (h % 4) * 128 + g * 32
                pb_t[g * 32:(g + 1) * 32, col:col + 32] = base[h]
    return cos_t, sin_t, pb_t.astype(ml_dtypes.bfloat16)


_NC_CACHE = {}
TRACE = False
REPEAT = 1
LAST_RESULT = None


def kernel(x, pos_bias, focus_present_mask, w_q, w_kv, w_out, inv_freq):
    x = np.asarray(x)
    mask = tuple(bool(v) for v in np.asarray(focus_present_mask))
    cos_t, sin_t, pb_t = _host_tables(pos_bias, focus_present_mask, inv_freq)

    wq_bf = (np.asarray(w_q, np.float32) * (DIM_HEAD ** -0.5)).astype(ml_dtypes.bfloat16)
    wkv_bf = np.asarray(w_kv, np.float32).astype(ml_dtypes.bfloat16)
    wo_bf = np.asarray(w_out, np.float32).astype(ml_dtypes.bfloat16)

    if (mask, REPEAT) not in _NC_CACHE:
        _NC_CACHE[(mask, REPEAT)] = build_nc(mask, repeat=REPEAT)
    nc = _NC_CACHE[(mask, REPEAT)]

    xs = x.reshape(B, N_CORES, HW_SHARD, N, DIM)
    in_maps = []
    for c in range(N_CORES):
        xc = np.ascontiguousarray(xs[:, c]).reshape(TOK, DIM).astype(ml_dtypes.bfloat16)
        xc = np.ascontiguousarray(
            xc.reshape(N_TILES, TILE_T, 4, 128).transpose(0, 3, 2, 1)
        ).reshape(TOK, DIM)
        in_maps.append(dict(
            x=xc, wq=wq_bf, wkv=wkv_bf, wo=wo_bf,
            cosb=cos_t, sinb=sin_t, pbias=pb_t,
        ))

    global LAST_RESULT
    res = run_bass_kernel_spmd(nc, in_maps, core_ids=list(range(N_CORES)), trace=TRACE)
    LAST_RESULT = res
    outs = [r["out"].reshape(B, HW_SHARD, N, DIM) for r in res.results]
    return np.concatenate(outs, axis=1).astype(np.float32)



# revision 4
# speedup vs baseline: 3.3859x; 3.3859x over previous
"""Trainium2 Bass kernel for temporal attention (nn_Attention_4423816315129).

Per 128-token tile (= 4 temporal groups x 32 positions, one batch):
  xT    <- one DMA load of host pre-transposed x (bf16, dim on partitions)
  q,k,v <- PE matmuls accumulating 4 dim-blocks (PSUM fp32)
  rope  <- DVE: t*cos + swap(t)*sin_signed (swap = reversed-pair AP view)
  qT,kT <- PE transposes (128x128 blocks), ACT copy to SBUF bf16
  sim   <- per head ONE full matmul (K=64, M=128, N=128): all (group,group')
           pairs; cross-group entries killed by a -1e30 bias added on PE via
           identity @ pb_full (also encodes pos_bias and the focus-present
           diagonal mask), so exp() zeroes them exactly
  softmax (no max-subtract; logits are small): ACT exp, DVE segmented row
           sums + reciprocal; division folded into attention weights
  attnT <- per head PE transpose of the (128x128) attention block
  y     <- per head matmul lhsT=attnT rhs=v slice -> y natural (128 x 512)
  yT    <- PE transposes, then out = yT @ w_out blocks, fp32 copy, DMA store

All matmuls write full-partition contiguous PSUM regions (partition-sliced
multi-matmul PSUM writes are fatal on this toolchain/HW).
Sharding: hw axis split across 8 cores, pure data parallel.
"""

import numpy as np
import ml_dtypes

import concourse.bass as bass
from concourse import bacc
import concourse.mybir as mybir
import concourse.tile as tile
from concourse.bass import ts
from concourse.masks import make_identity
from concourse.bass_utils import run_bass_kernel_spmd

HEADS = 8
DIM_HEAD = 64
B = 2
HW = 1024
N = 32
DIM = 512
N_CORES = 8
HW_SHARD = HW // N_CORES            # 128
TOK = B * HW_SHARD * N              # 8192 tokens per core
TILE_T = 128
N_TILES = TOK // TILE_T             # 64
BF16 = mybir.dt.bfloat16
F32 = mybir.dt.float32


def build_nc(mask_flags, n_tiles=N_TILES, repeat=1, stage='full'):
    nc = bacc.Bacc("TRN2", target_bir_lowering=False)

    x_d = nc.dram_tensor("x", [TOK, DIM], BF16, kind="ExternalInput")
    wq_d = nc.dram_tensor("wq", [DIM, DIM], BF16, kind="ExternalInput")
    wkv_d = nc.dram_tensor("wkv", [DIM, 2 * DIM], BF16, kind="ExternalInput")
    wo_d = nc.dram_tensor("wo", [DIM, DIM], BF16, kind="ExternalInput")
    cos_d = nc.dram_tensor("cosb", [TILE_T, DIM], BF16, kind="ExternalInput")
    sin_d = nc.dram_tensor("sinb", [TILE_T, DIM], BF16, kind="ExternalInput")
    pb_d = nc.dram_tensor("pbias", [TILE_T, B * 2048], BF16, kind="ExternalInput")
    out_d = nc.dram_tensor("out", [TOK, DIM], F32, kind="ExternalOutput")

    Exp = mybir.ActivationFunctionType.Exp
    Add = mybir.AluOpType.add

    with tile.TileContext(nc) as tc:
        with (
            tc.tile_pool(name="const", bufs=1) as cpool,
            tc.tile_pool(name="xt", bufs=N_TILES) as xtpool,
            tc.tile_pool(name="work", bufs=4) as wpool,
            tc.tile_pool(name="small", bufs=3) as spool,
            tc.tile_pool(name="ps", bufs=8, space="PSUM") as ps,
        ):
            wq_sb = cpool.tile([128, 4, DIM], BF16)
            nc.gpsimd.dma_start(wq_sb[:], wq_d.ap().rearrange("(kb p) c -> p kb c", p=128))
            wkv_sb = cpool.tile([128, 4, 2 * DIM], BF16)
            nc.gpsimd.dma_start(wkv_sb[:], wkv_d.ap().rearrange("(kb p) c -> p kb c", p=128))
            wo_sb = cpool.tile([128, 4, DIM], BF16)
            nc.gpsimd.dma_start(wo_sb[:], wo_d.ap().rearrange("(kb p) c -> p kb c", p=128))
            cos_sb = cpool.tile([TILE_T, DIM], BF16)
            nc.gpsimd.dma_start(cos_sb[:], cos_d.ap())
            sin_sb = cpool.tile([TILE_T, DIM], BF16)
            nc.gpsimd.dma_start(sin_sb[:], sin_d.ap())
            pb_sb = cpool.tile([TILE_T, B * 2048], BF16)
            nc.gpsimd.dma_start(pb_sb[:], pb_d.ap())
            ident = cpool.tile([128, 128], BF16)
            make_identity(nc, ident[:])

            def rope(dst, src_bf):
                t1 = wpool.tile([TILE_T, DIM], BF16, tag="rope_t1")
                nc.vector.tensor_mul(t1[:], src_bf[:], cos_sb[:])
                t2 = wpool.tile([TILE_T, DIM], BF16, tag="rope_t2")
                sw = src_bf[:].rearrange("p (c two) -> p c two", two=2)[:, :, ::-1]
                nc.vector.tensor_mul(
                    t2[:].rearrange("p (c two) -> p c two", two=2),
                    sw,
                    sin_sb[:].rearrange("p (c two) -> p c two", two=2),
                )
                nc.vector.tensor_add(dst[:], t1[:], t2[:])

            import contextlib
            rep_ctx = tc.For_i(0, repeat, 1) if repeat > 1 else contextlib.nullcontext()
            with rep_ctx:
              for tt in range(n_tiles):
                b = tt // max(1, n_tiles // B)

                xT = xtpool.tile([128, DIM], BF16)
                nc.sync.dma_start(xT[:], x_d.ap()[ts(tt, TILE_T), :])

                # ---- projections ----
                v_ps = ps.tile([TILE_T, DIM], F32, tag="ps")
                for kb in range(4):
                    nc.tensor.matmul(v_ps[:], xT[:, ts(kb, 128)], wkv_sb[:, kb, DIM:],
                                     start=(kb == 0), stop=(kb == 3))
                v_sb = wpool.tile([TILE_T, DIM], BF16, tag="v_sb")
                nc.scalar.copy(v_sb[:], v_ps[:])

                q_ps = ps.tile([TILE_T, DIM], F32, tag="ps")
                k_ps = ps.tile([TILE_T, DIM], F32, tag="ps")
                for kb in range(4):
                    nc.tensor.matmul(q_ps[:], xT[:, ts(kb, 128)], wq_sb[:, kb, :],
                                     start=(kb == 0), stop=(kb == 3))
                for kb in range(4):
                    nc.tensor.matmul(k_ps[:], xT[:, ts(kb, 128)], wkv_sb[:, kb, :DIM],
                                     start=(kb == 0), stop=(kb == 3))

                q_bf = wpool.tile([TILE_T, DIM], BF16, tag="q_bf")
                nc.scalar.copy(q_bf[:], q_ps[:])
                k_bf = wpool.tile([TILE_T, DIM], BF16, tag="k_bf")
                nc.scalar.copy(k_bf[:], k_ps[:])

                q_r = wpool.tile([TILE_T, DIM], BF16, tag="q_r")
                rope(q_r, q_bf)
                k_r = wpool.tile([TILE_T, DIM], BF16, tag="k_r")
                rope(k_r, k_bf)

                # ---- qT, kT per head: (64 d x 128 tok), base partition 0
                # (mixing matmul operand base partitions is fatal on this HW) ----
                qT_ps = ps.tile([64, 2 * DIM], BF16, tag="ps")
                kT_ps = ps.tile([64, 2 * DIM], BF16, tag="ps")
                for h in range(HEADS):
                    nc.tensor.transpose(qT_ps[:, ts(h, 128)], q_r[:, ts(h, 64)], ident[:])
                for h in range(HEADS):
                    nc.tensor.transpose(kT_ps[:, ts(h, 128)], k_r[:, ts(h, 64)], ident[:])
                qT_sb = wpool.tile([64, 2 * DIM], BF16, tag="qT_sb")
                nc.scalar.copy(qT_sb[:], qT_ps[:])
                kT_sb = wpool.tile([64, 2 * DIM], BF16, tag="kT_sb")
                nc.scalar.copy(kT_sb[:], kT_ps[:])

                # ---- sim: per head, all group pairs (128x128) ----
                sim_ps = [ps.tile([TILE_T, DIM], F32, tag="ps", name=f"sim_ps{i}") for i in range(2)]
                for h in range(HEADS):
                    nc.tensor.matmul(
                        sim_ps[h // 4][:, ts(h % 4, 128)],
                        qT_sb[:, ts(h, 128)],
                        kT_sb[:, ts(h, 128)],
                        start=(h % 4 == 0), stop=False, skip_group_check=True,
                    )
                # + pos_bias/mask via identity @ pb_full (kills cross-group)
                for h in range(HEADS):
                    cb = b * 2048 + (h // 4) * 1024 + (h % 4) * 128
                    nc.tensor.matmul(
                        sim_ps[h // 4][:, ts(h % 4, 128)],
                        ident[:],
                        pb_sb[:, cb:cb + 128],
                        start=False, stop=(h % 4 == 3), skip_group_check=True,
                    )

                if stage == 'sim':
                    o_sb = wpool.tile([TILE_T, DIM], F32, tag="o_sb")
                    nc.scalar.copy(o_sb[:], sim_ps[0][:])
                    nc.scalar.dma_start(out_d.ap()[ts(tt, TILE_T), :], o_sb[:])
                    continue
                # ---- softmax ----
                e_sb = wpool.tile([TILE_T, 2 * DIM], BF16, tag="e_sb")
                nc.scalar.activation(e_sb[:, :DIM], sim_ps[0][:], Exp)
                nc.scalar.activation(e_sb[:, DIM:], sim_ps[1][:], Exp)
                sums = spool.tile([TILE_T, HEADS], F32, tag="sums")
                nc.vector.tensor_reduce(
                    sums[:], e_sb[:].rearrange("p (h j) -> p h j", j=128),
                    axis=mybir.AxisListType.X, op=Add,
                )
                rcp = spool.tile([TILE_T, HEADS], F32, tag="rcp")
                nc.vector.reciprocal(rcp[:], sums[:])
                attn = wpool.tile([TILE_T, 2 * DIM], BF16, tag="attn")
                nc.vector.tensor_mul(
                    attn[:].rearrange("p (h j) -> p h j", j=128),
                    e_sb[:].rearrange("p (h j) -> p h j", j=128),
                    rcp[:].rearrange("p h -> p h ()").broadcast_to((TILE_T, HEADS, 128)),
                )

                if stage == 'attn':
                    o_sb = wpool.tile([TILE_T, DIM], F32, tag="o_sb")
                    nc.scalar.copy(o_sb[:], attn[:, :DIM])
                    nc.scalar.dma_start(out_d.ap()[ts(tt, TILE_T), :], o_sb[:])
                    continue
                # ---- attnT per head (PE transpose), then y = attnT.T @ v ----
                aT_ps = [ps.tile([TILE_T, DIM], BF16, tag="ps", name=f"aT_ps{i}") for i in range(2)]
                for h in range(HEADS):
                    nc.tensor.transpose(aT_ps[h // 4][:, ts(h % 4, 128)],
                                        attn[:, ts(h, 128)], ident[:])
                aT_sb = wpool.tile([TILE_T, 2 * DIM], BF16, tag="aT_sb")
                nc.scalar.copy(aT_sb[:, :DIM], aT_ps[0][:])
                nc.scalar.copy(aT_sb[:, DIM:], aT_ps[1][:])

                y_ps = ps.tile([TILE_T, DIM], F32, tag="ps")
                for h in range(HEADS):
                    nc.tensor.matmul(y_ps[:, ts(h, 64)],
                                     aT_sb[:, ts(h, 128)], v_sb[:, ts(h, 64)],
                                     start=(h == 0), stop=(h == 7), skip_group_check=True)
                if stage == 'y':
                    o_sb = wpool.tile([TILE_T, DIM], F32, tag="o_sb")
                    nc.scalar.copy(o_sb[:], y_ps[:])
                    nc.scalar.dma_start(out_d.ap()[ts(tt, TILE_T), :], o_sb[:])
                    continue
                y_sb = wpool.tile([TILE_T, DIM], BF16, tag="y_sb")
                nc.scalar.copy(y_sb[:], y_ps[:])

                yT_ps = ps.tile([TILE_T, DIM], BF16, tag="ps")
                for hb in range(4):
                    nc.tensor.transpose(yT_ps[:, ts(hb, 128)], y_sb[:, ts(hb, 128)], ident[:])
                yT_sb = wpool.tile([TILE_T, DIM], BF16, tag="yT_sb")
                nc.scalar.copy(yT_sb[:], yT_ps[:])

                # ---- out projection ----
                o_ps = ps.tile([TILE_T, DIM], F32, tag="ps")
                for hb in range(4):
                    nc.tensor.matmul(o_ps[:], yT_sb[:, ts(hb, 128)], wo_sb[:, hb, :],
                                     start=(hb == 0), stop=(hb == 3))
                o_sb = wpool.tile([TILE_T, DIM], F32, tag="o_sb")
                nc.scalar.copy(o_sb[:], o_ps[:])
                nc.scalar.dma_start(out_d.ap()[ts(tt, TILE_T), :], o_sb[:])

    nc.compile()
    return nc


def _host_tables(pos_bias, focus_present_mask, inv_freq):
    pos = np.arange(N, dtype=np.float32)
    freqs = pos[:, None] * np.asarray(inv_freq, np.float32)[None, :]
    fr2 = np.repeat(freqs, 2, axis=-1)
    cos = np.cos(fr2)
    sin = np.sin(fr2)
    sign = np.tile(np.array([-1.0, 1.0], np.float32), DIM_HEAD // 2)
    sin_signed = sin * sign[None, :]
    cos_t = np.tile(cos, (4, HEADS)).astype(ml_dtypes.bfloat16)
    sin_t = np.tile(sin_signed, (4, HEADS)).astype(ml_dtypes.bfloat16)

    pb = np.asarray(pos_bias, np.float32)
    mask = np.asarray(focus_present_mask)
    eye = np.eye(N, dtype=bool)
    # pb_full[(g,i), b*2048 + (h//4)*1024 + (h%4)*128 + g'*32 + j]
    pb_t = np.full((TILE_T, B * 2048), -1e30, np.float32)
    for b in range(B):
        base = pb.copy()                       # (H, 32, 32)
        if mask[b]:
            m = np.full_like(base, -1e30)
            m[:, eye] = base[:, eye]
            base = m
        for h in range(HEADS):
            for g in range(4):
                col = b * 2048 + (h // 4) * 1024 + (h % 4) * 128 + g * 32
                pb_t[g * 32:(g + 1) * 32, col:col + 32] = base[h]
    return cos_t, sin_t, pb_t.astype(ml_dtypes.bfloat16)


_NC_CACHE = {}
TRACE = False
REPEAT = 1
LAST_RESULT = None


def kernel(x, pos_bias, focus_present_mask, w_q, w_kv, w_out, inv_freq):
    x = np.asarray(x)
    mask = tuple(bool(v) for v in np.asarray(focus_present_mask))
    cos_t, sin_t, pb_t = _host_tables(pos_bias, focus_present_mask, inv_freq)

    wq_bf = (np.asarray(w_q, np.float32) * (DIM_HEAD ** -0.5)).astype(ml_dtypes.bfloat16)
    wkv_bf = np.asarray(w_kv, np.float32).astype(ml_dtypes.bfloat16)
    wo_bf = np.asarray(w_out, np.float32).astype(ml_dtypes.bfloat16)

    if (mask, REPEAT) not in _NC_CACHE:
        _NC_CACHE[(mask, REPEAT)] = build_nc(mask, repeat=REPEAT)
    nc = _NC_CACHE[(mask, REPEAT)]

    xs = x.reshape(B, N_CORES, HW_SHARD, N, DIM)
    in_maps = []
    for c in range(N_CORES):
        xc = np.ascontiguousarray(xs[:, c]).reshape(TOK, DIM).astype(ml_dtypes.bfloat16)
        xc = np.ascontiguousarray(
            xc.reshape(N_TILES, TILE_T, 4, 128).transpose(0, 3, 2, 1)
        ).reshape(TOK, DIM)
        in_maps.append(dict(
            x=xc, wq=wq_bf, wkv=wkv_bf, wo=wo_bf,
            cosb=cos_t, sinb=sin_t, pbias=pb_t,
        ))

    global LAST_RESULT
    res = run_bass_kernel_spmd(nc, in_maps, core_ids=list(range(N_CORES)), trace=TRACE)
    LAST_RESULT = res
    outs = [r["out"].reshape(B, HW_SHARD, N, DIM) for r in res.results]
    return np.concatenate(outs, axis=1).astype(np.float32)

